# revision 14
# baseline (speedup 1.0000x reference)
"""Biaffine kernel for Trainium2, data-parallel over batch across 8 NeuronCores.

Reference math (per batch b):
    Daug = [D, 1]                                  # [S, d+1]
    out  = Daug @ U @ H^T + (Daug @ W[:d+1])[:, None] + (H @ W[d+1:])[None, :]

Algebraic refactor used here (d = 1024):
    U0 = U[:d]                # [d, d]
    c  = U[d] + W[d+1:]       # [d]  (folds the ones-row of Daug and the H linear term)
    T' = D @ U0 + c           # [S, d]
    dlin = D @ W[:d] + W[d]   # [S]  (tiny; computed host-side)
    out  = T' @ H^T + dlin[:, None]

Device kernel per core (4 batches, 384 matmuls):
    matmul1: T'^T[j, x] = sum_k U0[k, j] * D^T[k, x]  (lhsT = U0, rhs = D^T)
             + per-partition bias c fused into the PSUM->SBUF copy (DVE/ACT)
    matmul2: out[x, y] = sum_j T'^T[j, x] * H^T[j, y] (lhsT = T'^T, rhs = H^T)
             + per-partition bias dlin fused into the PSUM->SBUF copy (DVE)

Performance structure (measured on trn2):
  - Matmul operands are float16 (fp32 PSUM accumulation); output stored fp16
    and upconverted host-side. PE issue rate at N=512 is ~216 ns/matmul
    (~2.37 GHz effective); 384 data matmuls = 83 us of PE floor. End-to-end
    rel err ~4.6e-4.
  - exec_time is measured from the framework's first preamble memset (~5.9 us
    into the trace, fixed) to the end of the NEFF semaphore-file wipe, which
    is a fixed ~8.4 us after the last DMA packet lands. So the objective is:
    last output byte as early as possible.
  - DMA hardware queues (sync=Q1, scalar=Q10) process one packet (= one
    per-partition contiguous run, capped 8 KB) per ~15.7 ns + ~0.57 ns/KB:
    1KB packets -> 65 GB/s, 2KB -> 122, 4KB -> 228, 8KB -> 415 GB/s per
    queue. Batch-0 loads therefore use coarse multi-kt chunks (2-8 KB rows)
    on both HW queues, with ht0 + bias columns on the gpsimd SWDGE queue,
    so all of batch 0 (4.2 MB) is on-chip by ~14 us and matmul1 is never
    DMA-paced after its first chunk (ready ~10 us).
  - The HAM matmul clock runs at half speed for the first ~5.3 us of PE
    activity (full at ~13 us). Warm-up matmuls on a memset tile (n=6,
    ~426 ns each cold) bridge the preamble->first-data window so the ramp
    burns no useful work.
  - Batch-0 matmul1: kt 0-3 bank-outer (paced by the first two DMA chunks),
    then per-bank kt 4-7 tail chains so the 8 PSUM-bank stops stagger
    ~864 ns apart; the bias copies alternate DVE/ACT and keep pace, so
    matmul2's first tile never waits on a copy. Batches 1-3 run jm-outer
    whole chains with DVE copies (stops 1.73 us apart).
  - Stores alternate rings (xt even: scalar, xt odd: sync). The final output
    tile is computed as a half + two quarter chains in separate PSUM banks;
    the very last 32 KB piece is stored partition-split (rows 0-63 scalar,
    64-127 sync) so its drain is ~1 us of 256 B packets instead of 2 us of
    128 B packets.

BIAFFINE_MM=f32r switches to fp32r matmuls; BIAFFINE_MM=f32 switches to
exact fp32 matmuls (~3x slower, rel err ~5e-7). BIAFFINE_U0_BF16=1 loads U0
as bf16 (mixed 16-bit operand dtypes work on the PE; same bytes, more error
— off by default).
"""
import os
import sys

import numpy as np

for _p in (
    "/root/.axon_site",
    "/root/.axon_site/_ro/trn_rl_repo",
    "/root/.axon_site/_ro/pypackages",
    "/opt/trn_rl_repo",
):
    if os.path.isdir(_p) and _p not in sys.path:
        sys.path.append(_p)

import concourse.bass as bass
import concourse.mybir as mybir
import concourse.tile as tile
from concourse import bacc
from concourse.bass_utils import run_bass_kernel_spmd

B, S, D_DIM = 32, 512, 1024
N_CORES = 8
BPC = B // N_CORES  # batches per core
KT = D_DIM // 128  # 8 k-tiles (contraction over d)
JT = D_DIM // 128  # 8 j-tiles (M dim of matmul1)
XT = S // 128  # 4 x-tiles (M dim of matmul2)

_NC_CACHE = {}


def _mode() -> str:
    m = os.environ.get("BIAFFINE_MM", "f16")
    assert m in ("f16", "bf16", "f32r", "f32"), m
    return m


def _build_nc(mode: str) -> bass.Bass:
    nc = bacc.Bacc()
    f32 = mybir.dt.float32
    mm_dt = {
        "f16": mybir.dt.float16,
        "bf16": mybir.dt.bfloat16,
        "f32r": mybir.dt.float32r,
        "f32": f32,
    }[mode]
    n_warm = int(os.environ.get("BIAFFINE_WARM", "12"))
    # Store the output in the matmul dtype (fp16/bf16): halves store traffic
    # and the drain-tail store; the host upconverts to fp32. Output range
    # (|out| < ~200) is far inside fp16 range; adds ~2.4e-4 quantization.
    out_dt = mm_dt if mode in ("f16", "bf16") else f32
    u0_dt = (
        mybir.dt.bfloat16
        if mode == "f16" and os.environ.get("BIAFFINE_U0_BF16", "0") == "1"
        else mm_dt
    )

    # Inputs arrive pre-swizzled to the SBUF layout: [.., p, kt, x] so each
    # partition's DMA read is one contiguous block.
    dt_in = nc.dram_tensor("dt_in", [BPC, 128, KT, S], mm_dt, kind="ExternalInput")
    ht_in = nc.dram_tensor("ht_in", [BPC, 128, KT, S], mm_dt, kind="ExternalInput")
    u0_in = nc.dram_tensor("u0_in", [128, KT, D_DIM], u0_dt, kind="ExternalInput")
    ccol_in = nc.dram_tensor("ccol_in", [128, JT], f32, kind="ExternalInput")
    dcol_in = nc.dram_tensor("dcol_in", [128, BPC * XT], f32, kind="ExternalInput")
    out_t = nc.dram_tensor("out", [BPC, S, S], out_dt, kind="ExternalOutput")

    with tile.TileContext(nc) as tc:
        with (
            tc.tile_pool(name="const", bufs=1) as cpool,
            tc.tile_pool(name="dh", bufs=2) as dh_pool,
            tc.tile_pool(name="tt", bufs=2) as tt_pool,
            tc.tile_pool(name="ot", bufs=3) as ot_pool,
            tc.tile_pool(name="ps", bufs=8, space="PSUM") as ps_pool,
        ):
            # HAM warm-up: a few matmuls on a memset tile fill the startup DMA
            # window with real array work so the PE is at the warm clock when
            # the first data matmul issues. (gpsimd memset: it's idle early.)
            warm_sb = cpool.tile([128, S], mm_dt, name="warm_sb")
            nc.gpsimd.memset(warm_sb[:], 0.0)
            warm_ps = ps_pool.tile([128, S], f32, tag="ps", bufs=8, name="warm_ps")
            for _ in range(n_warm):
                nc.tensor.matmul(
                    warm_ps[:], lhsT=warm_sb[:, :128], rhs=warm_sb[:], start=True,
                    stop=True,
                )
            # Quarter-length warmups: fine-grained bridge to the first data
            # matmul's gate (~12.5 us) — a PE idle gap here stalls the HAM
            # clock ramp (full speed needs ~5.4 us of uninterrupted PE
            # activity from the first matmul), costing ~3 us.
            for _ in range(int(os.environ.get("BIAFFINE_WARM_Q", "2"))):
                nc.tensor.matmul(
                    warm_ps[:, :128], lhsT=warm_sb[:, :128], rhs=warm_sb[:, :128],
                    start=True, stop=True,
                )

            # Batch-0 loads. The consumer of a chunk waits for that chunk's
            # whole DMA (per-instruction completion semaphore, +~0.9 us
            # propagation), and a queue retires ~one packet (= per-partition
            # contiguous run) per 16 ns + ~0.6 ns/KB. So: fine chunks early
            # (latency-gating), coarse chunks late (throughput), never a
            # chunk so coarse that its completion overruns its deadline.
            # mm1 kt step k consumes (u0[k], dt0[k]) at ~13+1.7k us; ht0[jm]
            # feeds mm2-b0 from ~20 us. ht0 splits between the gpsimd SWDGE
            # queue (h1) and the scalar queue after dt0 (h2).
            u0_t = cpool.tile([128, KT * D_DIM], u0_dt, name="u0_t")
            ccol = cpool.tile([128, JT], f32)
            dcol = cpool.tile([128, BPC * XT], f32)
            dt0 = dh_pool.tile([128, KT * S], mm_dt, tag="dtf", name="dtf0")
            ht0 = dh_pool.tile([128, KT * S], mm_dt, tag="htf", name="htf0")
            u0s = u0_in.rearrange("p k d -> p (k d)")
            dts0 = dt_in[0].rearrange("p k x -> p (k x)")
            hts0 = ht_in[0].rearrange("p k x -> p (k x)")
            # All the core's queues share ONE packet engine round-robin, so
            # concurrent chunks split the packet rate by count, and rate
            # scales with packet size. Fine FIRST chunks (kt0: both queues
            # balanced small => first data matmul ~11.5 us, no PE idle gap,
            # no HAM re-ramp), coarse LATER chunks (4-6 KB packets => the
            # rest of batch 0 lands by ~19 us, ahead of every consumer).
            # sync queue: u0 in pairs kt0-1 / kt2-3 / kt4-5 / kt6-7 (4 KB
            # packets), then b1 dt. The first pair + dt0's first pair gate
            # the first data matmul at ~12.3 us (packets done ~11, +~1.2 us
            # completion-semaphore lag).
            for c in range(4):
                nc.sync.dma_start(
                    u0_t[:, 2 * c * D_DIM : 2 * (c + 1) * D_DIM],
                    u0s[:, 2 * c * D_DIM : 2 * (c + 1) * D_DIM],
                )
            # scalar queue: dt0 kt0-1 / kt2-3 / kt4-7, then ht0 halves, b1 ht.
            nc.scalar.dma_start(dt0[:, : 2 * S], dts0[:, : 2 * S])
            nc.scalar.dma_start(dt0[:, 2 * S : 4 * S], dts0[:, 2 * S : 4 * S])
            nc.scalar.dma_start(dt0[:, 4 * S :], dts0[:, 4 * S :])
            hw0 = KT * S // 2
            nc.scalar.dma_start(ht0[:, :hw0], hts0[:, :hw0])
            nc.scalar.dma_start(ht0[:, hw0:], hts0[:, hw0:])
            # gpsimd SWDGE queue: only the tiny bias columns (anything big
            # here would steal round-robin packet slots from the critical
            # dt0/u0 chunks).
            nc.gpsimd.dma_start(ccol[:], ccol_in[:])
            nc.gpsimd.dma_start(dcol[:], dcol_in[:])

            def u0j(kt, jm):
                return u0_t[:, kt * D_DIM + jm * 128 : kt * D_DIM + (jm + 1) * 128]

            dt_full, ht_full = dt0, ht0
            for b in range(BPC):
                # Prefetch batch b+1 as one DMA per tensor (dt: sync ring,
                # ht: scalar ring); emitted before this batch's stores so the
                # loads aren't queued behind store-data-ready waits. For b=0
                # it is emitted AFTER matmul1 instead, so the ACT-engine tt
                # copies precede the ht prefetch on the scalar queue.
                nxt_dt, nxt_ht = None, None

                def prefetch_next(b):
                    nxt_dt = dh_pool.tile([128, KT * S], mm_dt, tag="dtf", name="dtf")
                    nxt_ht = dh_pool.tile([128, KT * S], mm_dt, tag="htf", name="htf")
                    dsrc = dt_in[b + 1].rearrange("p k x -> p (k x)")
                    hsrc = ht_in[b + 1].rearrange("p k x -> p (k x)")
                    hw = KT * S // 2
                    if b == 0:
                        # Batch 1 is consumed right on the heels of the
                        # startup loads: split it so the first half (kt 0-3)
                        # lands before batch-1 matmul1 reaches it (subtile
                        # deps let those matmuls start on the half).
                        nc.sync.dma_start(nxt_dt[:, :hw], dsrc[:, :hw])
                        nc.sync.dma_start(nxt_dt[:, hw:], dsrc[:, hw:])
                        nc.scalar.dma_start(nxt_ht[:, :hw], hsrc[:, :hw])
                        nc.scalar.dma_start(nxt_ht[:, hw:], hsrc[:, hw:])
                    else:
                        nc.sync.dma_start(nxt_dt[:], dsrc)
                        nc.scalar.dma_start(nxt_ht[:], hsrc)
                    return nxt_dt, nxt_ht

                if 0 < b < BPC - 1:
                    nxt_dt, nxt_ht = prefetch_next(b)

                dt_rhs = [dt_full[:, kt * S : (kt + 1) * S] for kt in range(KT)]
                ht_rhs = [ht_full[:, kt * S : (kt + 1) * S] for kt in range(KT)]

                # matmul1: T'^T[jm*128+p, x]  (+ bias c)
                tt_t = [
                    tt_pool.tile([128, S], mm_dt, tag=f"tt{jm}", name=f"tt{jm}")
                    for jm in range(JT)
                ]
                if b == 0:
                    # kt 0-3 bank-outer: each step needs only chunks already
                    # delivered (kt0-1 ready ~10 us, kt2-3 ~12.2 us), so the
                    # first data matmul starts while the HAM clock is still
                    # ramping. Then per-bank kt4-7 tail chains stagger the 8
                    # bank stops ~864 ns apart so the alternating DVE/ACT
                    # bias copies (~740 ns each) drain banks in lockstep and
                    # matmul2 never waits.
                    ps_l = [
                        ps_pool.tile([128, S], f32, tag="ps", bufs=8, name=f"ps{jm}")
                        for jm in range(JT)
                    ]
                    for kt in range(5):
                        for jm in range(JT):
                            nc.tensor.matmul(
                                ps_l[jm][:],
                                lhsT=u0j(kt, jm),
                                rhs=dt_rhs[kt],
                                start=(kt == 0),
                                stop=False,
                            )
                    for jm in range(JT):
                        for kt in range(5, KT):
                            nc.tensor.matmul(
                                ps_l[jm][:],
                                lhsT=u0j(kt, jm),
                                rhs=dt_rhs[kt],
                                start=False,
                                stop=(kt == KT - 1),
                            )
                        # DVE-only copies: the ~864 ns bank-stop stagger
                        # outpaces the ~740 ns copy, so no second copy
                        # engine (and no ACT table load in the preamble,
                        # which would delay the scalar queue's first DMA).
                        nc.vector.tensor_scalar_add(
                            tt_t[jm][:], ps_l[jm][:], ccol[:, jm : jm + 1]
                        )
                    nxt_dt, nxt_ht = prefetch_next(0)
                else:
                    for jm in range(JT):
                        ps = ps_pool.tile([128, S], f32, tag="ps", bufs=8, name="ps")
                        for kt in range(KT):
                            nc.tensor.matmul(
                                ps[:],
                                lhsT=u0j(kt, jm),
                                rhs=dt_rhs[kt],
                                start=(kt == 0),
                                stop=(kt == KT - 1),
                            )
                        nc.vector.tensor_scalar_add(
                            tt_t[jm][:], ps[:], ccol[:, jm : jm + 1]
                        )

                # matmul2: out[xt*128+p, y]  (+ bias dlin)
                for xt in range(XT):
                    po = ps_pool.tile([128, S], f32, tag="ps", bufs=8, name="po")
                    for step, jm in enumerate(range(JT)):
                        nc.tensor.matmul(
                            po[:],
                            lhsT=tt_t[jm][:, xt * 128 : (xt + 1) * 128],
                            rhs=ht_rhs[jm],
                            start=(step == 0),
                            stop=(step == JT - 1),
                        )
                    ot = ot_pool.tile([128, S], out_dt, tag="ot", name="ot")
                    nc.vector.tensor_scalar_add(
                        ot[:], po[:], dcol[:, b * XT + xt : b * XT + xt + 1]
                    )
                    if b < BPC - 1:
                        # Alternate store rings to balance bytes.
                        eng = nc.scalar if xt % 2 == 0 else nc.sync
                        eng.dma_start(out_t[b, xt * 128 : (xt + 1) * 128, :], ot[:])
                    else:
                        # Last batch: split every tile's store by PARTITIONS
                        # across both rings. The queue (not the engine) waits
                        # on data-ready, so each ring drains 64 x 1 KB
                        # packets (~1 us) per tile with no FIFO backlog, and
                        # the final tile's last byte leaves ~1.8 us after the
                        # last matmul (vs ~2.3 us for column-split pieces —
                        # total packets scale with pieces, so one chain +
                        # one copy + two half-partition stores is optimal).
                        nc.scalar.dma_start(
                            out_t[b, xt * 128 : xt * 128 + 64, :], ot[:64, :]
                        )
                        nc.sync.dma_start(
                            out_t[b, xt * 128 + 64 : (xt + 1) * 128, :], ot[64:, :]
                        )

                if nxt_dt is not None:
                    dt_full, ht_full = nxt_dt, nxt_ht
    nc.finalize()
    return nc


def _get_nc() -> bass.Bass:
    key = f"nc_{_mode()}"
    if key not in _NC_CACHE:
        _NC_CACHE[key] = _build_nc(_mode())
    return _NC_CACHE[key]


def _round_fp32r(a: np.ndarray) -> np.ndarray:
    """Round fp32 to fp32r layout: RNE to 11-bit mantissa, low 12 bits zero."""
    bits = np.ascontiguousarray(a, dtype=np.float32).view(np.uint32)
    odd = (bits >> 12) & np.uint32(1)
    out = (bits + np.uint32(0x7FF) + odd) & np.uint32(0xFFFFF000)
    return out.view(np.float32)


def kernel(D, H, U, W):
    D = np.ascontiguousarray(np.asarray(D, dtype=np.float32))
    H = np.ascontiguousarray(np.asarray(H, dtype=np.float32))
    U = np.asarray(U, dtype=np.float32)
    W = np.asarray(W, dtype=np.float32)
    d = D_DIM
    mode = _mode()
    np_mm = np.dtype(
        mybir.dt.np(
            {
                "f16": mybir.dt.float16,
                "bf16": mybir.dt.bfloat16,
                "f32r": mybir.dt.float32r,
                "f32": mybir.dt.float32,
            }[mode]
        )
    )

    def to_mm(a: np.ndarray) -> np.ndarray:
        if mode == "f32r":
            return _round_fp32r(a)
        return np.ascontiguousarray(a).astype(np_mm)

    # U0 swizzled to [128, KT, d]: [p, kt, j] = U0[kt*128+p, j]
    U0 = np.ascontiguousarray(U[:d, :].reshape(KT, 128, d).transpose(1, 0, 2))
    if mode == "f16" and os.environ.get("BIAFFINE_U0_BF16", "0") == "1":
        U0 = U0.astype(np.dtype(mybir.dt.np(mybir.dt.bfloat16)))
    else:
        U0 = to_mm(U0)
    c = (U[d, :] + W[d + 1 :]).astype(np.float32)  # [d]
    # ccol[p, jm] = c[jm*128 + p]
    ccol = np.ascontiguousarray(c.reshape(JT, 128).T)
    # dlin[b, x] = D[b, x] . W[:d] + W[d]  (from unrounded fp32 D: exact)
    dlin = (D @ W[:d] + W[d]).astype(np.float32)  # [B, S]

    in_maps = []
    for cidx in range(N_CORES):
        sl = slice(cidx * BPC, (cidx + 1) * BPC)
        # [b, p, kt, x] = X[b, x, kt*128+p]  (transpose + swizzle in one copy)
        Dt = to_mm(D[sl].reshape(BPC, S, KT, 128).transpose(0, 3, 2, 1))
        Ht = to_mm(H[sl].reshape(BPC, S, KT, 128).transpose(0, 3, 2, 1))
        # dcol[p, b*XT + xt] = dlin[b, xt*128 + p]
        dcol = np.ascontiguousarray(
            dlin[sl].reshape(BPC, XT, 128).transpose(2, 0, 1).reshape(128, BPC * XT)
        )
        in_maps.append(
            {
                "dt_in": Dt,
                "ht_in": Ht,
                "u0_in": U0,
                "ccol_in": ccol,
                "dcol_in": dcol,
            }
        )

    nc = _get_nc()
    trace = bool(int(os.environ.get("BIAFFINE_TRACE", "0")))
    kwargs = {}
    if trace:
        tdir = os.environ.get("BIAFFINE_TRACE_DIR")
        if tdir:
            os.makedirs(tdir, exist_ok=True)
            kwargs["tmpdir"] = tdir
    res = run_bass_kernel_spmd(
        nc, in_maps, core_ids=list(range(N_CORES)), trace=trace, **kwargs
    )
    if trace and res.exec_time_ns is not None:
        print(f"HW exec time: {res.exec_time_ns} ns")

    out = np.concatenate([res.results[i]["out"] for i in range(N_CORES)], axis=0)
    return np.ascontiguousarray(out.astype(np.float32))


# revision 16
# speedup vs baseline: 1.0403x; 1.0403x over previous
"""Biaffine kernel for Trainium2, data-parallel over batch across 8 NeuronCores.

Reference math (per batch b):
    Daug = [D, 1]                                  # [S, d+1]
    out  = Daug @ U @ H^T + (Daug @ W[:d+1])[:, None] + (H @ W[d+1:])[None, :]

Algebraic refactor used here (d = 1024):
    U0 = U[:d]                # [d, d]
    c  = U[d] + W[d+1:]       # [d]  (folds the ones-row of Daug and the H linear term)
    T' = D @ U0 + c           # [S, d]
    dlin = D @ W[:d] + W[d]   # [S]  (tiny; computed host-side)
    out  = T' @ H^T + dlin[:, None]

Device kernel per core (4 batches, 384 matmuls):
    matmul1: T'^T[j, x] = sum_k U0[k, j] * D^T[k, x]  (lhsT = U0, rhs = D^T)
             + per-partition bias c fused into the PSUM->SBUF copy (DVE/ACT)
    matmul2: out[x, y] = sum_j T'^T[j, x] * H^T[j, y] (lhsT = T'^T, rhs = H^T)
             + per-partition bias dlin fused into the PSUM->SBUF copy (DVE)

Performance structure (measured on trn2):
  - Matmul operands are float16 (fp32 PSUM accumulation); output stored fp16
    and upconverted host-side. PE issue rate at N=512 is ~216 ns/matmul
    (~2.37 GHz effective); 384 data matmuls = 83 us of PE floor. End-to-end
    rel err ~4.6e-4.
  - exec_time is measured from the framework's first preamble memset (~5.9 us
    into the trace, fixed) to the end of the NEFF semaphore-file wipe, which
    is a fixed ~8.4 us after the last DMA packet lands. So the objective is:
    last output byte as early as possible.
  - DMA hardware queues (sync=Q1, scalar=Q10) process one packet (= one
    per-partition contiguous run, capped 8 KB) per ~15.7 ns + ~0.57 ns/KB:
    1KB packets -> 65 GB/s, 2KB -> 122, 4KB -> 228, 8KB -> 415 GB/s per
    queue. Batch-0 loads therefore use coarse multi-kt chunks (2-8 KB rows)
    on both HW queues, with ht0 + bias columns on the gpsimd SWDGE queue,
    so all of batch 0 (4.2 MB) is on-chip by ~14 us and matmul1 is never
    DMA-paced after its first chunk (ready ~10 us).
  - The HAM matmul clock runs at half speed for the first ~5.3 us of PE
    activity (full at ~13 us). Warm-up matmuls on a memset tile (n=6,
    ~426 ns each cold) bridge the preamble->first-data window so the ramp
    burns no useful work.
  - Batch-0 matmul1: kt 0-3 bank-outer (paced by the first two DMA chunks),
    then per-bank kt 4-7 tail chains so the 8 PSUM-bank stops stagger
    ~864 ns apart; the bias copies alternate DVE/ACT and keep pace, so
    matmul2's first tile never waits on a copy. Batches 1-3 run jm-outer
    whole chains with DVE copies (stops 1.73 us apart).
  - Stores alternate rings (xt even: scalar, xt odd: sync). The final output
    tile is computed as a half + two quarter chains in separate PSUM banks;
    the very last 32 KB piece is stored partition-split (rows 0-63 scalar,
    64-127 sync) so its drain is ~1 us of 256 B packets instead of 2 us of
    128 B packets.

BIAFFINE_MM=f32r switches to fp32r matmuls; BIAFFINE_MM=f32 switches to
exact fp32 matmuls (~3x slower, rel err ~5e-7). BIAFFINE_U0_BF16=1 loads U0
as bf16 (mixed 16-bit operand dtypes work on the PE; same bytes, more error
— off by default).
"""
import os
import sys

import numpy as np

for _p in (
    "/root/.axon_site",
    "/root/.axon_site/_ro/trn_rl_repo",
    "/root/.axon_site/_ro/pypackages",
    "/opt/trn_rl_repo",
):
    if os.path.isdir(_p) and _p not in sys.path:
        sys.path.append(_p)

import concourse.bass as bass
import concourse.mybir as mybir
import concourse.tile as tile
from concourse import bacc
from concourse.bass_utils import run_bass_kernel_spmd

B, S, D_DIM = 32, 512, 1024
N_CORES = 8
BPC = B // N_CORES  # batches per core
KT = D_DIM // 128  # 8 k-tiles (contraction over d)
JT = D_DIM // 128  # 8 j-tiles (M dim of matmul1)
XT = S // 128  # 4 x-tiles (M dim of matmul2)

_NC_CACHE = {}


def _mode() -> str:
    m = os.environ.get("BIAFFINE_MM", "f16")
    assert m in ("f16", "bf16", "f32r", "f32"), m
    return m


def _build_nc(mode: str) -> bass.Bass:
    nc = bacc.Bacc()
    f32 = mybir.dt.float32
    mm_dt = {
        "f16": mybir.dt.float16,
        "bf16": mybir.dt.bfloat16,
        "f32r": mybir.dt.float32r,
        "f32": f32,
    }[mode]
    n_warm = int(os.environ.get("BIAFFINE_WARM", "14"))
    # Store the output in the matmul dtype (fp16/bf16): halves store traffic
    # and the drain-tail store; the host upconverts to fp32. Output range
    # (|out| < ~200) is far inside fp16 range; adds ~2.4e-4 quantization.
    out_dt = mm_dt if mode in ("f16", "bf16") else f32
    u0_dt = (
        mybir.dt.bfloat16
        if mode == "f16" and os.environ.get("BIAFFINE_U0_BF16", "0") == "1"
        else mm_dt
    )

    # Inputs arrive pre-swizzled to the SBUF layout: [.., p, kt, x] so each
    # partition's DMA read is one contiguous block.
    dt_in = nc.dram_tensor("dt_in", [BPC, 128, KT, S], mm_dt, kind="ExternalInput")
    ht_in = nc.dram_tensor("ht_in", [BPC, 128, KT, S], mm_dt, kind="ExternalInput")
    u0_in = nc.dram_tensor("u0_in", [128, KT, D_DIM], u0_dt, kind="ExternalInput")
    ccol_in = nc.dram_tensor("ccol_in", [128, JT], f32, kind="ExternalInput")
    dcol_in = nc.dram_tensor("dcol_in", [128, BPC * XT], f32, kind="ExternalInput")
    out_t = nc.dram_tensor("out", [BPC, S, S], out_dt, kind="ExternalOutput")

    with tile.TileContext(nc) as tc:
        with (
            tc.tile_pool(name="const", bufs=1) as cpool,
            tc.tile_pool(name="dh", bufs=2) as dh_pool,
            tc.tile_pool(name="tt", bufs=2) as tt_pool,
            tc.tile_pool(name="ot", bufs=3) as ot_pool,
            tc.tile_pool(name="ps", bufs=8, space="PSUM") as ps_pool,
        ):
            # HAM warm-up: a few matmuls on a memset tile fill the startup DMA
            # window with real array work so the PE is at the warm clock when
            # the first data matmul issues. (gpsimd memset: it's idle early.)
            warm_sb = cpool.tile([128, S], mm_dt, name="warm_sb")
            nc.gpsimd.memset(warm_sb[:], 0.0)
            warm_ps = ps_pool.tile([128, S], f32, tag="ps", bufs=8, name="warm_ps")
            for _ in range(n_warm):
                nc.tensor.matmul(
                    warm_ps[:], lhsT=warm_sb[:, :128], rhs=warm_sb[:], start=True,
                    stop=True,
                )
            # Quarter-length warmups: fine-grained bridge to the first data
            # matmul's gate (~12.5 us) — a PE idle gap here stalls the HAM
            # clock ramp (full speed needs ~5.4 us of uninterrupted PE
            # activity from the first matmul), costing ~3 us.
            for _ in range(int(os.environ.get("BIAFFINE_WARM_Q", "2"))):
                nc.tensor.matmul(
                    warm_ps[:, :128], lhsT=warm_sb[:, :128], rhs=warm_sb[:, :128],
                    start=True, stop=True,
                )

            # Batch-0 loads. The consumer of a chunk waits for that chunk's
            # whole DMA (per-instruction completion semaphore, +~0.9 us
            # propagation), and a queue retires ~one packet (= per-partition
            # contiguous run) per 16 ns + ~0.6 ns/KB. So: fine chunks early
            # (latency-gating), coarse chunks late (throughput), never a
            # chunk so coarse that its completion overruns its deadline.
            # mm1 kt step k consumes (u0[k], dt0[k]) at ~13+1.7k us; ht0[jm]
            # feeds mm2-b0 from ~20 us. ht0 splits between the gpsimd SWDGE
            # queue (h1) and the scalar queue after dt0 (h2).
            u0_t = cpool.tile([128, KT * D_DIM], u0_dt, name="u0_t")
            ccol = cpool.tile([128, JT], f32)
            dcol = cpool.tile([128, BPC * XT], f32)
            dt0 = dh_pool.tile([128, KT * S], mm_dt, tag="dtf", name="dtf0")
            ht0 = dh_pool.tile([128, KT * S], mm_dt, tag="htf", name="htf0")
            u0s = u0_in.rearrange("p k d -> p (k d)")
            dts0 = dt_in[0].rearrange("p k x -> p (k x)")
            hts0 = ht_in[0].rearrange("p k x -> p (k x)")
            # All the core's queues share ONE packet engine round-robin, so
            # concurrent chunks split the packet rate by count, and rate
            # scales with packet size. Fine FIRST chunks (kt0: both queues
            # balanced small => first data matmul ~11.5 us, no PE idle gap,
            # no HAM re-ramp), coarse LATER chunks (4-6 KB packets => the
            # rest of batch 0 lands by ~19 us, ahead of every consumer).
            # Phase-matched pairs: each phase puts EQUAL packet counts on
            # both HW queues (the shared packet engine serves queues
            # round-robin, so a mismatched pair starves the smaller-packet
            # queue). Phase 1 covers kt0 AND kt1 on both tensors, so the
            # single gate (~13.4 us incl. the ~1.2 us completion-semaphore
            # lag) feeds the first TWO kt-steps and every later chunk
            # arrives a full step ahead of its consumer.
            # sync queue: u0 kt0-1 / kt2-3 / kt4-5 / kt6-7, then b1 dt.
            for c in range(4):
                nc.sync.dma_start(
                    u0_t[:, 2 * c * D_DIM : 2 * (c + 1) * D_DIM],
                    u0s[:, 2 * c * D_DIM : 2 * (c + 1) * D_DIM],
                )
            # scalar queue: dt0 kt0-1 / kt2-3 / kt4-7, then ht0 halves, b1 ht.
            nc.scalar.dma_start(dt0[:, : 2 * S], dts0[:, : 2 * S])
            nc.scalar.dma_start(dt0[:, 2 * S : 4 * S], dts0[:, 2 * S : 4 * S])
            nc.scalar.dma_start(dt0[:, 4 * S :], dts0[:, 4 * S :])
            hw0 = KT * S // 2
            nc.scalar.dma_start(ht0[:, :hw0], hts0[:, :hw0])
            nc.scalar.dma_start(ht0[:, hw0:], hts0[:, hw0:])
            # gpsimd SWDGE queue: only the tiny bias columns (anything big
            # here would steal round-robin packet slots from the critical
            # dt0/u0 chunks).
            nc.gpsimd.dma_start(ccol[:], ccol_in[:])
            nc.gpsimd.dma_start(dcol[:], dcol_in[:])

            def u0j(kt, jm):
                return u0_t[:, kt * D_DIM + jm * 128 : kt * D_DIM + (jm + 1) * 128]

            dt_full, ht_full = dt0, ht0
            for b in range(BPC):
                # Prefetch batch b+1 as one DMA per tensor (dt: sync ring,
                # ht: scalar ring); emitted before this batch's stores so the
                # loads aren't queued behind store-data-ready waits. For b=0
                # it is emitted AFTER matmul1 instead, so the ACT-engine tt
                # copies precede the ht prefetch on the scalar queue.
                nxt_dt, nxt_ht = None, None

                def prefetch_next(b):
                    nxt_dt = dh_pool.tile([128, KT * S], mm_dt, tag="dtf", name="dtf")
                    nxt_ht = dh_pool.tile([128, KT * S], mm_dt, tag="htf", name="htf")
                    dsrc = dt_in[b + 1].rearrange("p k x -> p (k x)")
                    hsrc = ht_in[b + 1].rearrange("p k x -> p (k x)")
                    hw = KT * S // 2
                    if b == 0:
                        # Batch 1 is consumed right on the heels of the
                        # startup loads: split it so the first half (kt 0-3)
                        # lands before batch-1 matmul1 reaches it (subtile
                        # deps let those matmuls start on the half).
                        nc.sync.dma_start(nxt_dt[:, :hw], dsrc[:, :hw])
                        nc.sync.dma_start(nxt_dt[:, hw:], dsrc[:, hw:])
                        nc.scalar.dma_start(nxt_ht[:, :hw], hsrc[:, :hw])
                        nc.scalar.dma_start(nxt_ht[:, hw:], hsrc[:, hw:])
                    else:
                        nc.sync.dma_start(nxt_dt[:], dsrc)
                        nc.scalar.dma_start(nxt_ht[:], hsrc)
                    return nxt_dt, nxt_ht

                if 0 < b < BPC - 1:
                    nxt_dt, nxt_ht = prefetch_next(b)

                dt_rhs = [dt_full[:, kt * S : (kt + 1) * S] for kt in range(KT)]
                ht_rhs = [ht_full[:, kt * S : (kt + 1) * S] for kt in range(KT)]

                # matmul1: T'^T[jm*128+p, x]  (+ bias c)
                tt_t = [
                    tt_pool.tile([128, S], mm_dt, tag=f"tt{jm}", name=f"tt{jm}")
                    for jm in range(JT)
                ]
                if b == 0:
                    # kt 0-3 bank-outer: each step needs only chunks already
                    # delivered (kt0-1 ready ~10 us, kt2-3 ~12.2 us), so the
                    # first data matmul starts while the HAM clock is still
                    # ramping. Then per-bank kt4-7 tail chains stagger the 8
                    # bank stops ~864 ns apart so the alternating DVE/ACT
                    # bias copies (~740 ns each) drain banks in lockstep and
                    # matmul2 never waits.
                    ps_l = [
                        ps_pool.tile([128, S], f32, tag="ps", bufs=8, name=f"ps{jm}")
                        for jm in range(JT)
                    ]
                    for kt in range(5):
                        for jm in range(JT):
                            nc.tensor.matmul(
                                ps_l[jm][:],
                                lhsT=u0j(kt, jm),
                                rhs=dt_rhs[kt],
                                start=(kt == 0),
                                stop=False,
                            )
                    for jm in range(JT):
                        for kt in range(5, KT):
                            nc.tensor.matmul(
                                ps_l[jm][:],
                                lhsT=u0j(kt, jm),
                                rhs=dt_rhs[kt],
                                start=False,
                                stop=(kt == KT - 1),
                            )
                        # DVE-only copies: the ~864 ns bank-stop stagger
                        # outpaces the ~740 ns copy, so no second copy
                        # engine (and no ACT table load in the preamble,
                        # which would delay the scalar queue's first DMA).
                        nc.vector.tensor_scalar_add(
                            tt_t[jm][:], ps_l[jm][:], ccol[:, jm : jm + 1]
                        )
                    nxt_dt, nxt_ht = prefetch_next(0)
                else:
                    for jm in range(JT):
                        ps = ps_pool.tile([128, S], f32, tag="ps", bufs=8, name="ps")
                        for kt in range(KT):
                            nc.tensor.matmul(
                                ps[:],
                                lhsT=u0j(kt, jm),
                                rhs=dt_rhs[kt],
                                start=(kt == 0),
                                stop=(kt == KT - 1),
                            )
                        nc.vector.tensor_scalar_add(
                            tt_t[jm][:], ps[:], ccol[:, jm : jm + 1]
                        )

                # matmul2: out[xt*128+p, y]  (+ bias dlin)
                for xt in range(XT):
                    po = ps_pool.tile([128, S], f32, tag="ps", bufs=8, name="po")
                    for step, jm in enumerate(range(JT)):
                        nc.tensor.matmul(
                            po[:],
                            lhsT=tt_t[jm][:, xt * 128 : (xt + 1) * 128],
                            rhs=ht_rhs[jm],
                            start=(step == 0),
                            stop=(step == JT - 1),
                        )
                    ot = ot_pool.tile([128, S], out_dt, tag="ot", name="ot")
                    nc.vector.tensor_scalar_add(
                        ot[:], po[:], dcol[:, b * XT + xt : b * XT + xt + 1]
                    )
                    if b < BPC - 1:
                        # Alternate store rings to balance bytes.
                        eng = nc.scalar if xt % 2 == 0 else nc.sync
                        eng.dma_start(out_t[b, xt * 128 : (xt + 1) * 128, :], ot[:])
                    else:
                        # Last batch: split every tile's store by PARTITIONS
                        # across both rings. The queue (not the engine) waits
                        # on data-ready, so each ring drains 64 x 1 KB
                        # packets (~1 us) per tile with no FIFO backlog, and
                        # the final tile's last byte leaves ~1.8 us after the
                        # last matmul (vs ~2.3 us for column-split pieces —
                        # total packets scale with pieces, so one chain +
                        # one copy + two half-partition stores is optimal).
                        nc.scalar.dma_start(
                            out_t[b, xt * 128 : xt * 128 + 64, :], ot[:64, :]
                        )
                        nc.sync.dma_start(
                            out_t[b, xt * 128 + 64 : (xt + 1) * 128, :], ot[64:, :]
                        )

                if nxt_dt is not None:
                    dt_full, ht_full = nxt_dt, nxt_ht
    nc.finalize()
    return nc


def _get_nc() -> bass.Bass:
    key = f"nc_{_mode()}"
    if key not in _NC_CACHE:
        _NC_CACHE[key] = _build_nc(_mode())
    return _NC_CACHE[key]


def _round_fp32r(a: np.ndarray) -> np.ndarray:
    """Round fp32 to fp32r layout: RNE to 11-bit mantissa, low 12 bits zero."""
    bits = np.ascontiguousarray(a, dtype=np.float32).view(np.uint32)
    odd = (bits >> 12) & np.uint32(1)
    out = (bits + np.uint32(0x7FF) + odd) & np.uint32(0xFFFFF000)
    return out.view(np.float32)


def kernel(D, H, U, W):
    D = np.ascontiguousarray(np.asarray(D, dtype=np.float32))
    H = np.ascontiguousarray(np.asarray(H, dtype=np.float32))
    U = np.asarray(U, dtype=np.float32)
    W = np.asarray(W, dtype=np.float32)
    d = D_DIM
    mode = _mode()
    np_mm = np.dtype(
        mybir.dt.np(
            {
                "f16": mybir.dt.float16,
                "bf16": mybir.dt.bfloat16,
                "f32r": mybir.dt.float32r,
                "f32": mybir.dt.float32,
            }[mode]
        )
    )

    def to_mm(a: np.ndarray) -> np.ndarray:
        if mode == "f32r":
            return _round_fp32r(a)
        return np.ascontiguousarray(a).astype(np_mm)

    # U0 swizzled to [128, KT, d]: [p, kt, j] = U0[kt*128+p, j]
    U0 = np.ascontiguousarray(U[:d, :].reshape(KT, 128, d).transpose(1, 0, 2))
    if mode == "f16" and os.environ.get("BIAFFINE_U0_BF16", "0") == "1":
        U0 = U0.astype(np.dtype(mybir.dt.np(mybir.dt.bfloat16)))
    else:
        U0 = to_mm(U0)
    c = (U[d, :] + W[d + 1 :]).astype(np.float32)  # [d]
    # ccol[p, jm] = c[jm*128 + p]
    ccol = np.ascontiguousarray(c.reshape(JT, 128).T)
    # dlin[b, x] = D[b, x] . W[:d] + W[d]  (from unrounded fp32 D: exact)
    dlin = (D @ W[:d] + W[d]).astype(np.float32)  # [B, S]

    in_maps = []
    for cidx in range(N_CORES):
        sl = slice(cidx * BPC, (cidx + 1) * BPC)
        # [b, p, kt, x] = X[b, x, kt*128+p]  (transpose + swizzle in one copy)
        Dt = to_mm(D[sl].reshape(BPC, S, KT, 128).transpose(0, 3, 2, 1))
        Ht = to_mm(H[sl].reshape(BPC, S, KT, 128).transpose(0, 3, 2, 1))
        # dcol[p, b*XT + xt] = dlin[b, xt*128 + p]
        dcol = np.ascontiguousarray(
            dlin[sl].reshape(BPC, XT, 128).transpose(2, 0, 1).reshape(128, BPC * XT)
        )
        in_maps.append(
            {
                "dt_in": Dt,
                "ht_in": Ht,
                "u0_in": U0,
                "ccol_in": ccol,
                "dcol_in": dcol,
            }
        )

    nc = _get_nc()
    trace = bool(int(os.environ.get("BIAFFINE_TRACE", "0")))
    kwargs = {}
    if trace:
        tdir = os.environ.get("BIAFFINE_TRACE_DIR")
        if tdir:
            os.makedirs(tdir, exist_ok=True)
            kwargs["tmpdir"] = tdir
    res = run_bass_kernel_spmd(
        nc, in_maps, core_ids=list(range(N_CORES)), trace=trace, **kwargs
    )
    if trace and res.exec_time_ns is not None:
        print(f"HW exec time: {res.exec_time_ns} ns")

    out = np.concatenate([res.results[i]["out"] for i in range(N_CORES)], axis=0)
    return np.ascontiguousarray(out.astype(np.float32))
